# revision 14
# baseline (speedup 1.0000x reference)
"""Trainium2 Bass kernel for nn_AngleFreqEnhance.

out = x + ifft2(ifftshift(gain * fftshift(fft2(x, ortho)))), ortho).real
    = ifft2(G_total * fft2(x))  with  G_total = 1 + sym(ifftshift(gain))
(sym() is the Hermitian symmetrization (G(k)+G(-k))/2, which is exact here
because the output only keeps the real part.)

Strategy (8 NeuronCores, data-parallel over the 1024 independent (b,c)
slices, 128 slices per core):

- Pair trick: two real slices are processed as one complex field
  z = x1 + i*x2.  Because G_total is real and even, each enhanced field is
  exactly real, so out1 = Re(result), out2 = Im(result).  Halves the work.
- 2D FFT as dense DFT matmuls on the TensorEngine (bf16, 1 col/cycle),
  4 stages, alternating data-as-lhsT ("flip") and matrix-as-lhsT
  ("noflip") so the contracted axis is always on partitions: NO transposes.
    S1 flip   : U^T[w,u] = z^T C (+/- z^T S terms)        contract h
    S2 noflip : V^T[v,u] = C^T U^T (+/- S^T U^T)          contract w
    gain      : W^T = G_total^T * V^T  (VectorE, on PSUM read)
    S3 flip   : P[u,w'] = W^T^T C (+/- ... S)             contract v
    S4 noflip : Z[h,w'] = C^T P (+/- S^T P)               contract u
- The gain is computed on device from angle_weights via 9 constant basis
  masks (the geometry is data-independent): G_total = B0 + sum_a w_a * B_a.
- PSUM accumulation folds all complex-arithmetic adds into the PE.
"""

import math

import numpy as np

N = 256
N_ANGLES = 8
HIGH_FREQ_RATIO = 0.3
N_CORES = 8
SLICES_PER_CORE = 128  # 4*256 / 8


# ----------------------------------------------------------------------------
# Host-side constants
# ----------------------------------------------------------------------------

def _host_constants():
    jk = np.arange(N, dtype=np.float64)
    ang = 2.0 * math.pi * np.outer(jk, jk) / N
    C = np.cos(ang)
    S = np.sin(ang)
    CS = np.concatenate([C, S], axis=1)  # [256, 512]
    cs = np.stack([CS[0:128], CS[128:256]]).astype(np.float32)  # [2,128,512]
    sneg = np.stack([(-S)[0:128], (-S)[128:256]]).astype(np.float32)

    # gain basis, [u,v] freq layout, then transposed to [v,u] and column-
    # duplicated so one [128,512] tile serves both pairs of a quad.
    cy = cx = N // 2
    y = np.arange(N, dtype=np.float64)[:, None] - cy
    x = np.arange(N, dtype=np.float64)[None, :] - cx
    r = np.sqrt(y * y + x * x)
    theta = np.arctan2(y, x) + math.pi
    astep = 2.0 * math.pi / N_ANGLES
    aidx = np.mod(np.floor(theta / astep).astype(np.int64), N_ANGLES)
    hf = r > HIGH_FREQ_RATIO * min(cy, cx)

    rev = (-np.arange(N)) % N

    def sym_shift(M):
        M0 = np.fft.ifftshift(M)
        return 0.5 * (M0 + M0[rev][:, rev])

    scale = 1.0 / (N * N)
    basis_uv = np.zeros((9, N, N), dtype=np.float64)
    basis_uv[0] = (1.0 + sym_shift((~hf).astype(np.float64))) * scale
    for a in range(N_ANGLES):
        basis_uv[1 + a] = sym_shift((hf & (aidx == a)).astype(np.float64)) * scale

    basis = np.zeros((9, 2, 128, 512), dtype=np.float32)
    for j in range(9):
        bT = basis_uv[j].T  # [v, u]
        for k in range(2):
            chunk = bT[128 * k:128 * (k + 1)]  # [128, 256]
            basis[j, k] = np.concatenate([chunk, chunk], axis=1)
    return cs, sneg, basis


# ----------------------------------------------------------------------------
# Device kernel builder
# ----------------------------------------------------------------------------

def build_kernel(n_slices=SLICES_PER_CORE, debug=False, enable_asserts=False,
                 dt_name="f32r"):
    import concourse.bass as bass  # noqa: F401
    import concourse.tile as tile
    from concourse import bacc, mybir
    from contextlib import ExitStack

    F32 = mybir.dt.float32
    DT = {"f32r": mybir.dt.float32r, "bf16": mybir.dt.bfloat16,
          "f32": mybir.dt.float32}[dt_name]
    MUL = mybir.AluOpType.mult
    ADD = mybir.AluOpType.add

    nc = bacc.Bacc(
        "TRN2",
        target_bir_lowering=False,
        debug=debug,
        enable_asserts=enable_asserts,
        num_devices=N_CORES,
    )

    xa = nc.declare_dram_parameter("x", [n_slices, N, N], DT, isOutput=False).ap()
    awa = nc.declare_dram_parameter("aw", [N_ANGLES], F32, isOutput=False).ap()
    csa = nc.declare_dram_parameter("cs", [2, 128, 512], DT, isOutput=False).ap()
    snega = nc.declare_dram_parameter("sneg", [2, 128, 256], DT, isOutput=False).ap()
    basisa = nc.declare_dram_parameter("basis", [9, 2, 128, 512], F32, isOutput=False).ap()
    outa = nc.declare_dram_parameter("out", [n_slices, N, N], F32, isOutput=True).ap()

    assert n_slices % 4 == 0
    n_quads = n_slices // 4

    def mm(ps, lhsT, rhs, start, stop):
        nc.tensor.matmul(ps, lhsT, rhs, start=start, stop=stop)

    with ExitStack() as ctx:
        tc = ctx.enter_context(tile.TileContext(nc))
        cpool = ctx.enter_context(tc.tile_pool(name="const", bufs=1))

        cs_sb = []
        sneg_sb = []
        gain_sb = []
        # constants ride the gpsimd DMA queue so the sync queue's first
        # descriptors are the quad-0 data loads (saves ~3us of head stall)
        for k in range(2):
            t = cpool.tile([128, 512], DT, tag=f"cs{k}", name=f"cs{k}")
            nc.gpsimd.dma_start(t[:], csa[k])
            cs_sb.append(t)
            t = cpool.tile([128, 256], DT, tag=f"sneg{k}", name=f"sneg{k}")
            nc.gpsimd.dma_start(t[:], snega[k])
            sneg_sb.append(t)
            gain_sb.append(cpool.tile([128, 512], DT, tag=f"gain{k}", name=f"gain{k}"))

        # main pools are created FIRST so their SBUF addresses are below
        # and disjoint from the gain-setup scratch: when the setup pool
        # closes, nothing reuses its space (avoids a WAR stall on the
        # first z-loads observed when setup space was recycled).
        zin = ctx.enter_context(tc.tile_pool(name="zin", bufs=64))
        utp = ctx.enter_context(tc.tile_pool(name="ut", bufs=32))
        wtp = ctx.enter_context(tc.tile_pool(name="wt", bufs=12))
        ppool = ctx.enter_context(tc.tile_pool(name="pp", bufs=12))
        outp = ctx.enter_context(tc.tile_pool(name="outp", bufs=12))
        fps = ctx.enter_context(tc.tile_pool(name="fps", bufs=4, space="PSUM"))
        nps = ctx.enter_context(tc.tile_pool(name="nps", bufs=4, space="PSUM"))

        # --- PE pre-warm: ~3.5us of dummy matmuls during the DMA-fill
        # head so the HAM clock gate reaches K=8/8 (2.4 GHz) before the
        # first real matmul instead of ~6us into the stream ---
        warm = cpool.tile([128, 128], DT, tag="warm", name="warm")
        nc.gpsimd.memset(warm[:], 0.0)
        wps = fps.tile([128, 128], F32, tag="fps", name="ps")
        for i in range(34):
            nc.tensor.matmul(wps[:], warm[:], warm[:],
                             start=(i == 0), stop=(i == 33))
        wsink = cpool.tile([128, 128], F32, tag="wsink", name="wsink")
        nc.scalar.copy(wsink[:], wps[:])

        # --- one-time gain computation: G^T = B0 + sum_a w_a * B_a ---
        with tc.tile_pool(name="gsetup", bufs=10) as gp:
            aw_sb = gp.tile([1, N_ANGLES], F32, tag="aw", name="aw_sb")
            nc.gpsimd.dma_start(aw_sb[:], awa[None, :])
            ones = gp.tile([1, 128], F32, tag="ones", name="ones_sb")
            nc.gpsimd.memset(ones[:], 1.0)
            awb_ps = fps.tile([128, N_ANGLES], F32, tag="fps", name="awb_ps")
            nc.tensor.matmul(awb_ps[:], ones[:], aw_sb[:], start=True, stop=True)
            awb = gp.tile([128, N_ANGLES], F32, tag="awb", name="awb")
            nc.scalar.copy(awb[:], awb_ps[:])
            for k in range(2):
                acc = gp.tile([128, 256], F32, tag="bacc", name="bacc")
                nc.gpsimd.dma_start(acc[:], basisa[0, k, :, 0:256])
                for j in range(1, 9):
                    bt = gp.tile([128, 256], F32, tag="bt", name="bt")
                    nc.gpsimd.dma_start(bt[:], basisa[j, k, :, 0:256])
                    nxt = gp.tile([128, 256], F32, tag="bacc", name="bacc")
                    nc.vector.scalar_tensor_tensor(
                        nxt[:], bt[:], awb[:, j - 1:j], acc[:], MUL, ADD)
                    acc = nxt
                nc.scalar.copy(gain_sb[k][:, 0:256], acc[:])
                nc.scalar.copy(gain_sb[k][:, 256:512], acc[:])

        C0, C1 = cs_sb[0][:, 0:256], cs_sb[1][:, 0:256]
        S0, S1 = cs_sb[0][:, 256:512], cs_sb[1][:, 256:512]

        def emit_s1(q):
            """Loads + stage-1 for quad q; returns (z, ut) tile dicts."""
            sl = [4 * q + i for i in range(4)]
            z = {}
            for p in range(2):
                for comp in range(2):
                    s = sl[2 * p + comp]
                    for k in range(2):
                        t = zin.tile([128, 256], DT, tag="z", name="z")
                        nc.sync.dma_start(t[:], xa[s, 128 * k:128 * (k + 1), :])
                        z[p, comp, k] = t
            ut = {(c, k): utp.tile([128, 512], DT, tag="ut", name="ut")
                  for c in range(2) for k in range(2)}
            for p in range(2):
                for m in range(2):
                    ms = slice(128 * m, 128 * (m + 1))
                    hp = slice(256 * p, 256 * (p + 1))
                    ps = fps.tile([128, 256], F32, tag="fps", name="ps")
                    mm(ps[:], z[p, 0, 0][:, ms], C0, True, False)
                    mm(ps[:], z[p, 0, 1][:, ms], C1, False, False)
                    mm(ps[:], z[p, 1, 0][:, ms], S0, False, False)
                    mm(ps[:], z[p, 1, 1][:, ms], S1, False, True)
                    nc.vector.tensor_copy(ut[0, m][:, hp], ps[:])
                    ps = fps.tile([128, 256], F32, tag="fps", name="ps")
                    mm(ps[:], z[p, 1, 0][:, ms], C0, True, False)
                    mm(ps[:], z[p, 1, 1][:, ms], C1, False, False)
                    mm(ps[:], z[p, 0, 0][:, ms], sneg_sb[0][:], False, False)
                    mm(ps[:], z[p, 0, 1][:, ms], sneg_sb[1][:], False, True)
                    nc.vector.tensor_copy(ut[1, m][:, hp], ps[:])
            return ut

        def emit_rest(q, ut):
            sl = [4 * q + i for i in range(4)]
            # ---- S2 (noflip) + gain:  wt[comp][vchunk] [128,512] ----
            wt = {(c, k): wtp.tile([128, 512], DT, tag="wt", name="wt")
                  for c in range(2) for k in range(2)}
            for m in range(2):
                ms = slice(128 * m, 128 * (m + 1))
                ps = nps.tile([128, 512], F32, tag="nps", name="ps")
                mm(ps[:], C0[:, ms], ut[0, 0][:], True, False)
                mm(ps[:], C1[:, ms], ut[0, 1][:], False, False)
                mm(ps[:], S0[:, ms], ut[1, 0][:], False, False)
                mm(ps[:], S1[:, ms], ut[1, 1][:], False, True)
                nc.vector.tensor_mul(wt[0, m][:], ps[:], gain_sb[m][:])
                ps = nps.tile([128, 512], F32, tag="nps", name="ps")
                mm(ps[:], C0[:, ms], ut[1, 0][:], True, False)
                mm(ps[:], C1[:, ms], ut[1, 1][:], False, False)
                mm(ps[:], sneg_sb[0][:, ms], ut[0, 0][:], False, False)
                mm(ps[:], sneg_sb[1][:, ms], ut[0, 1][:], False, True)
                nc.vector.tensor_mul(wt[1, m][:], ps[:], gain_sb[m][:])

            # ---- S3 (flip): P[u,w'];  P[comp][uchunk] [128,512] ----
            P = {(c, k): ppool.tile([128, 512], DT, tag="pp", name="pt")
                 for c in range(2) for k in range(2)}
            for p in range(2):
                for m in range(2):
                    us = slice(256 * p + 128 * m, 256 * p + 128 * (m + 1))
                    hp = slice(256 * p, 256 * (p + 1))
                    ps = fps.tile([128, 256], F32, tag="fps", name="ps")
                    mm(ps[:], wt[0, 0][:, us], C0, True, False)
                    mm(ps[:], wt[0, 1][:, us], C1, False, False)
                    mm(ps[:], wt[1, 0][:, us], sneg_sb[0][:], False, False)
                    mm(ps[:], wt[1, 1][:, us], sneg_sb[1][:], False, True)
                    nc.scalar.copy(P[0, m][:, hp], ps[:])
                    ps = fps.tile([128, 256], F32, tag="fps", name="ps")
                    mm(ps[:], wt[0, 0][:, us], S0, True, False)
                    mm(ps[:], wt[0, 1][:, us], S1, False, False)
                    mm(ps[:], wt[1, 0][:, us], C0, False, False)
                    mm(ps[:], wt[1, 1][:, us], C1, False, True)
                    nc.scalar.copy(P[1, m][:, hp], ps[:])

            # ---- S4 (noflip): Z[h,w'];  ot[comp][hblock] [128,512] ----
            ot = {(c, m): outp.tile([128, 512], F32, tag="ot", name="ot")
                  for c in range(2) for m in range(2)}
            for m in range(2):
                ms = slice(128 * m, 128 * (m + 1))
                ps = nps.tile([128, 512], F32, tag="nps", name="ps")
                mm(ps[:], C0[:, ms], P[0, 0][:], True, False)
                mm(ps[:], C1[:, ms], P[0, 1][:], False, False)
                mm(ps[:], sneg_sb[0][:, ms], P[1, 0][:], False, False)
                mm(ps[:], sneg_sb[1][:, ms], P[1, 1][:], False, True)
                nc.scalar.copy(ot[0, m][:], ps[:])
                ps = nps.tile([128, 512], F32, tag="nps", name="ps")
                mm(ps[:], C0[:, ms], P[1, 0][:], True, False)
                mm(ps[:], C1[:, ms], P[1, 1][:], False, False)
                mm(ps[:], S0[:, ms], P[0, 0][:], False, False)
                mm(ps[:], S1[:, ms], P[0, 1][:], False, True)
                nc.scalar.copy(ot[1, m][:], ps[:])

            for p in range(2):
                for c in range(2):
                    for m in range(2):
                        nc.gpsimd.dma_start(
                            outa[sl[2 * p + c], 128 * m:128 * (m + 1), :],
                            ot[c, m][:, 256 * p:256 * (p + 1)])

        # software pipeline: PD quads of S1 run ahead so the PE has work
        # while the gain chain completes, and stays ahead of S2 afterwards
        PD = min(7, n_quads)
        uts = [emit_s1(q) for q in range(PD)]
        for q in range(n_quads):
            if q + PD < n_quads:
                uts.append(emit_s1(q + PD))
            emit_rest(q, uts[q])

    nc.compile()
    return nc


# ----------------------------------------------------------------------------
# Entry point
# ----------------------------------------------------------------------------

_CACHE = {}
DT_NAME = "bf16"


def _np_dt(dt_name):
    if dt_name == "bf16":
        import ml_dtypes
        return np.dtype(ml_dtypes.bfloat16)
    return np.dtype(np.float32)


def _get_nc(n_slices):
    key = (n_slices, DT_NAME)
    if key not in _CACHE:
        _CACHE[key] = build_kernel(n_slices, dt_name=DT_NAME)
    return _CACHE[key]


def kernel(x: np.ndarray, angle_weights: np.ndarray, _trace=False) -> np.ndarray:
    from concourse.bass_utils import run_bass_kernel_spmd

    B, Cc, H, W = x.shape
    assert (H, W) == (N, N)
    n_total = B * Cc
    per_core = n_total // N_CORES
    ndt = _np_dt(DT_NAME)
    xs = np.ascontiguousarray(x.reshape(n_total, H, W).astype(ndt))
    aw = np.ascontiguousarray(angle_weights.astype(np.float32))

    cs, sneg, basis = _host_constants()
    cs = cs.astype(ndt)
    sneg = sneg.astype(ndt)
    nc = _get_nc(per_core)

    in_maps = []
    for i in range(N_CORES):
        in_maps.append({
            "x": xs[i * per_core:(i + 1) * per_core],
            "aw": aw,
            "cs": cs,
            "sneg": sneg,
            "basis": basis,
        })

    try:
        res = run_bass_kernel_spmd(
            nc, in_maps, core_ids=list(range(N_CORES)), trace=_trace)
    except Exception:
        # transient NRT device errors have been observed; retry once
        res = run_bass_kernel_spmd(
            nc, in_maps, core_ids=list(range(N_CORES)), trace=_trace)
    outs = [res.results[i]["out"] for i in range(N_CORES)]
    out = np.concatenate(outs, axis=0).reshape(B, Cc, H, W)
    if _trace:
        kernel._last = res
    return out


if __name__ == "__main__":
    rng = np.random.default_rng(0)
    x = rng.standard_normal((4, 256, N, N)).astype(np.float32)
    aw = (1.0 + 0.1 * rng.standard_normal(N_ANGLES)).astype(np.float32)
    out = kernel(x, aw)
    print(out.shape, out.dtype)
